# revision 9
# baseline (speedup 1.0000x reference)
"""Cross-attention layer (vision<->text) on 8 Trainium2 NeuronCores.

Problem: B=16, Sv=St=1024, D=1024, fp32.
  q = vision @ Wq.T + bq            [B,Sv,D]
  k = text   @ Wk.T + bk            [B,St,D]
  v = text   @ Wv.T + bv            [B,St,D]
  scores = q @ k.T / sqrt(D)        [B,Sv,St]
  attn = softmax(scores, -1)
  cross_vision = attn @ v           [B,Sv,D]
  cross_text   = attn.T @ vision    [B,St,D]

Sharding: pure data-parallel over batch, 2 items per core, no collectives.

Algebraic restructure (q and k are never outputs):
  scores = vision @ A @ text.T + u 1^T + 1 w^T + c, with
    A = Wq.T @ Wk / sqrt(D)   (host-precomputed, fp64)
    u[s] = vision @ Wq.T @ bk / sqrt(D),  w[t] = text @ Wk.T @ bq / sqrt(D)
  The u[s] + c terms are constant along the softmax axis t, so they cancel
  exactly in softmax.  Only w[t] survives; host precomputes w = text @
  (Wk.T bq)/sqrt(D) and the device folds it in as the per-partition bias of
  the exp() (scores are built t-on-partitions).  bv is added on the host
  after gather (attn rows sum to 1, exact).  This removes one full GEMM per
  item (Q-proj + K-proj + scores -> A-proj + scores).

Per-core kernel design (per batch item; all matmuls float32r, full PE rate):
  prep:  PE-transpose text -> TT[d,t], vision -> VT[d,s]   (128 transposes)
  Vproj: V[t,d'] = TT.T @ wvt, TT tile stationary -> natural [t,d'] layout
         (no transposes needed, unlike a weight-stationary Vv^T approach)
  Hproj: HT[e,t] = A2-columns.T @ TT   (A2 = A.T streamed per 128-col block)
  ST:    ST[t,s] = HT.T @ VT per 128-row t-tile; E2 = exp(ST + w[t]) straight
         out of PSUM on ACT (scores are O(+-6), fp32 exp needs no max-sub)
  P5:    per s-tile: PE-transpose E2 column block -> E2T[s,t] blocks, evac'd
         with accum_out giving row sums -> rinv; E2T scaled by rinv in place
         (-> attn.T rows); CV[s,:] = E2.T @ V accumulated over t-tiles,
         scaled by rinv at PSUM evac
  CT:    cross_text = E2T_normalized.T @ vision accumulated over s-tiles,
         raw vision streamed back in, 8 concurrent PSUM groups
"""

import sys

import numpy as np

if "/opt/trn_rl_repo" not in sys.path:
    sys.path.insert(0, "/opt/trn_rl_repo")

import concourse.bass as bass
import concourse.tile as tile
from concourse import bacc
from concourse import mybir

PHASE_MARKS = []  # (phase_name, first_unused_instruction_id) at each boundary

P = 128
B, SEQ, DIM = 16, 1024, 1024
N_CORES = 8
BPC = B // N_CORES  # batch items per core
NT = DIM // P  # 8 tiles of 128 along d/e
F32 = mybir.dt.float32
F32R = mybir.dt.float32r
AF = mybir.ActivationFunctionType
H = 512  # half of a seq dim / PSUM-bank-sized chunk


def _emit(tc, ident, vis_d, txt_d, a2_d, wvt_d, wvt_sb, wcol_d, cv_d, ct_d,
          pools, b):
    nc = tc.nc

    def mark(name):
        nid = nc._state.next_id()
        PHASE_MARKS.append((f"b{b}_{name}", nid))

    (p_tt, p_vt, p_ht, p_v, p_wc, p_in, p_cvs, p_cts, p_vts,
     p_rp, p_rv, pp_t, pp_mm) = pools

    # per-item softmax bias column (w[t] rearranged t -> [ti, tt])
    wcol_sb = p_rv.tile([P, NT], F32, name="wcol_sb", tag="wcol")
    nc.sync.dma_start(out=wcol_sb,
                      in_=wcol_d[b].rearrange("(tt ti) -> ti tt", ti=P))

    def prep(src_d, dst, eng):
        """Transpose the full [SEQ, DIM] tensor into dst[d_in, d_out, seq].

        Each [128,1024] staging tile is filled by two half DMAs so the first
        transpose group only waits on 256KB (subtile deps).
        """
        for l in range(NT):
            tin = p_in.tile([P, DIM], F32R, name="tin", tag="xin")
            for hh in range(2):
                eng.dma_start(
                    out=tin[:, hh * H:(hh + 1) * H],
                    in_=src_d[b, l * P:(l + 1) * P, hh * H:(hh + 1) * H].bitcast(F32R))
            for tg in range(2):
                tp4 = pp_t.tile([P, 4, P], F32R, name="tp4", tag="tp4")
                for j in range(4):
                    c = tg * 4 + j
                    nc.tensor.matmul(
                        tp4[:, j, :], tin[:, c * P:(c + 1) * P], ident,
                        is_transpose=True, start=(j == 0), stop=(j == 3),
                        skip_group_check=True,
                    )
                if tg == 0:
                    nc.vector.tensor_copy(dst[:, 0:4, l * P:(l + 1) * P], tp4)
                else:
                    nc.scalar.copy(dst[:, 4:8, l * P:(l + 1) * P], tp4)

    # ---- prep both activations (DMAs race ahead on dual queues) ----
    mark("prep")
    tt_sb = p_tt.tile([P, NT, SEQ], F32R, name="tt_sb", tag="tt_e2")
    vt_sb = p_vt.tile([P, NT, SEQ], F32R, name="vt_sb", tag="vt")
    prep(txt_d, tt_sb, nc.sync)
    prep(vis_d, vt_sb, nc.scalar)
    if b == 0:
        # Wv.T load rides the sync queue BEHIND item 0's text tiles: it is
        # not needed until projV (~90us in), and putting it first would
        # starve the first prep transposes of DMA bandwidth.
        nc.sync.dma_start(
            out=wvt_sb,
            in_=wvt_d.rearrange("(do di) e -> di do e", di=P))

    # ---- HT[e,t] = A2-cols.T @ TT (A2 column block streamed per eo) ----
    mark("projH")
    ht_sb = p_ht.tile([P, NT, SEQ], F32R, name="ht_sb", tag="ht_e2t")
    for eo in range(NT):
        wc = p_wc.tile([P, NT, P], F32R, name="wc", tag="wc")
        nc.gpsimd.dma_start(
            out=wc,
            in_=a2_d[:, eo * P:(eo + 1) * P].rearrange("(do di) e -> di do e", di=P),
        )
        psh = [pp_mm.tile([P, H], F32, name=f"ps_h{i}", tag="mm") for i in range(2)]
        for do in range(NT):
            for th in range(2):
                nc.tensor.matmul(psh[th], wc[:, do, :],
                                 tt_sb[:, do, th * H:(th + 1) * H],
                                 start=(do == 0), stop=(do == NT - 1))
        for th in range(2):
            if th == 0:
                nc.vector.tensor_copy(ht_sb[:, eo, th * H:(th + 1) * H], psh[th])
            else:
                nc.scalar.copy(ht_sb[:, eo, th * H:(th + 1) * H], psh[th])

    # ---- V[t,d'] = TT.T @ wvt (TT tile stationary, wvt moving from SBUF) ----
    mark("projV")
    v_sb = p_v.tile([P, NT, SEQ], F32R, name="v_sb", tag="v")
    for tt in range(NT):
        psv = [pp_mm.tile([P, H], F32, name=f"ps_v{i}", tag="mm") for i in range(2)]
        for do in range(NT):
            for dc in range(2):
                nc.tensor.matmul(psv[dc], tt_sb[:, do, tt * P:(tt + 1) * P],
                                 wvt_sb[:, do, dc * H:(dc + 1) * H],
                                 start=(do == 0), stop=(do == NT - 1))
        for dc in range(2):
            if dc == 0:
                nc.vector.tensor_copy(v_sb[:, tt, dc * H:(dc + 1) * H], psv[dc])
            else:
                nc.scalar.copy(v_sb[:, tt, dc * H:(dc + 1) * H], psv[dc])

    # ---- ST[t,s] = HT.T @ VT; E2 = exp(ST + w[t]) (t on partitions) ----
    mark("ST")
    e2_sb = p_tt.tile([P, NT, SEQ], F32R, name="e2_sb", tag="tt_e2")
    for tt in range(NT):
        pst = [pp_mm.tile([P, H], F32, name=f"ps_s{i}", tag="mm") for i in range(2)]
        for eo in range(NT):
            for sh in range(2):
                nc.tensor.matmul(pst[sh], ht_sb[:, eo, tt * P:(tt + 1) * P],
                                 vt_sb[:, eo, sh * H:(sh + 1) * H],
                                 start=(eo == 0), stop=(eo == NT - 1))
        for sh in range(2):
            nc.scalar.activation(out=e2_sb[:, tt, sh * H:(sh + 1) * H], in_=pst[sh],
                                 func=AF.Exp, bias=wcol_sb[:, tt:tt + 1])

    # ---- P5: per s-tile: E2T blocks + row sums -> rinv; normalize; CV ----
    mark("P5")
    e2t_sb = p_ht.tile([P, NT, SEQ], F32R, name="e2t_sb", tag="ht_e2t")
    rinv = p_rv.tile([P, NT], F32, name="rinv", tag="rinv")
    for so in range(NT):
        rp = p_rp.tile([P, 2], F32, name="rp", tag="rp")
        for tg in range(2):
            tp4 = pp_t.tile([P, 4, P], F32R, name="tp4e", tag="tp4")
            for j in range(4):
                tt = tg * 4 + j
                nc.tensor.matmul(tp4[:, j, :], e2_sb[:, tt, so * P:(so + 1) * P],
                                 ident, is_transpose=True, start=(j == 0),
                                 stop=(j == 3), skip_group_check=True)
            nc.scalar.activation(out=e2t_sb[:, so, tg * H:(tg + 1) * H],
                                 in_=tp4, func=AF.Identity,
                                 accum_out=rp[:, tg:tg + 1])
        rsum = p_rp.tile([P, 1], F32, name="rsum", tag="rsum")
        nc.vector.tensor_add(rsum, rp[:, 0:1], rp[:, 1:2])
        nc.vector.reciprocal(rinv[:, so:so + 1], rsum)

        # normalize this E2T row-block in place -> attn.T rows (for CT)
        nc.vector.tensor_scalar_mul(e2t_sb[:, so, :], e2t_sb[:, so, :],
                                    scalar1=rinv[:, so:so + 1])

        # cross_vision[s-tile] = rinv * (E2.T @ V)
        cvs = p_cvs.tile([P, DIM], F32, name="cvs", tag="cvs")
        pcv = [pp_mm.tile([P, H], F32, name=f"ps_cv{i}", tag="mm") for i in range(2)]
        for tt in range(NT):
            for dc in range(2):
                nc.tensor.matmul(pcv[dc], e2_sb[:, tt, so * P:(so + 1) * P],
                                 v_sb[:, tt, dc * H:(dc + 1) * H],
                                 start=(tt == 0), stop=(tt == NT - 1))
        for dc in range(2):
            nc.scalar.mul(cvs[:, dc * H:(dc + 1) * H], pcv[dc],
                          mul=rinv[:, so:so + 1])
        nc.gpsimd.dma_start(out=cv_d[b, so * P:(so + 1) * P, :], in_=cvs)

    # ---- CT: cross_text = E2T_norm.T @ vision (vision streamed back in) ----
    mark("CT")
    for dc in range(2):
        pss = [pp_mm.tile([P, H], F32, name=f"ps_ct{i}", tag="mm") for i in range(6)]
        pss += [pp_t.tile([P, H], F32, name=f"ps_ct{i + 6}", tag="tp4") for i in range(2)]
        for so in range(NT):
            vtl = p_vts.tile([P, H], F32R, name="vtl", tag="vtl")
            eng = nc.sync if so % 2 == 0 else nc.scalar
            eng.dma_start(out=vtl,
                          in_=vis_d[b, so * P:(so + 1) * P, dc * H:(dc + 1) * H].bitcast(F32R))
            for tt in range(NT):
                nc.tensor.matmul(pss[tt], e2t_sb[:, so, tt * P:(tt + 1) * P], vtl,
                                 start=(so == 0), stop=(so == NT - 1))
        for tt in range(NT):
            cts = p_cts.tile([P, H], F32, name="cts", tag="cts")
            if tt % 2 == 0:
                nc.vector.tensor_copy(cts, pss[tt])
            else:
                nc.scalar.copy(cts, pss[tt])
            nc.gpsimd.dma_start(out=ct_d[b, tt * P:(tt + 1) * P, dc * H:(dc + 1) * H],
                                in_=cts)
    mark("end")


def build_nc():
    nc = bacc.Bacc("TRN2", target_bir_lowering=False, debug=False, num_devices=N_CORES)
    vis = nc.dram_tensor("vision", [BPC, SEQ, DIM], F32, kind="ExternalInput").ap()
    txt = nc.dram_tensor("text", [BPC, SEQ, DIM], F32, kind="ExternalInput").ap()
    a2_d = nc.dram_tensor("a2", [DIM, DIM], F32R, kind="ExternalInput").ap()
    wvt_d = nc.dram_tensor("wvt", [DIM, DIM], F32R, kind="ExternalInput").ap()
    wcol_d = nc.dram_tensor("wcol", [BPC, SEQ], F32, kind="ExternalInput").ap()
    id_d = nc.dram_tensor("ident128", [P, P], F32R, kind="ExternalInput").ap()
    cv_d = nc.dram_tensor("cross_vision", [BPC, SEQ, DIM], F32, kind="ExternalOutput").ap()
    ct_d = nc.dram_tensor("cross_text", [BPC, SEQ, DIM], F32, kind="ExternalOutput").ap()

    with tile.TileContext(nc) as tc:
        import contextlib
        with contextlib.ExitStack() as ctx:
            def sp(name, bufs):
                return ctx.enter_context(tc.tile_pool(name=name, bufs=bufs))

            p_tt = sp("tt", 1)    # TT then E2 (disjoint lifetimes)
            p_vt = sp("vt", 1)    # VT
            p_ht = sp("ht", 1)    # HT then E2T (disjoint lifetimes)
            p_v = sp("v", 1)      # V
            p_wvt = sp("wvt", 1)  # persistent Wv.T
            p_wc = sp("wc", 2)    # A2 column blocks
            p_in = sp("xin", 4)   # prep [128,1024] staging
            p_cvs = sp("cvs", 2)
            p_cts = sp("cts", 3)
            p_vts = sp("vtl", 4)  # CT vision tiles
            p_rp = sp("rp", 4)
            p_rv = sp("rv", 2)
            p_sm = sp("sm", 1)
            pp_t = ctx.enter_context(
                tc.tile_pool(name="pp_t", bufs=2, space=bass.MemorySpace.PSUM))
            pp_mm = ctx.enter_context(
                tc.tile_pool(name="pp_mm", bufs=6, space=bass.MemorySpace.PSUM))

            ident = p_sm.tile([P, P], F32R, name="ident")
            nc.sync.dma_start(out=ident, in_=id_d)
            wvt_sb = p_wvt.tile([P, NT, DIM], F32R, name="wvt_sb")

            pools = (p_tt, p_vt, p_ht, p_v, p_wc, p_in, p_cvs, p_cts, p_vts,
                     p_rp, p_rv, pp_t, pp_mm)
            for b in range(BPC):
                _emit(tc, ident, vis, txt, a2_d, wvt_d, wvt_sb, wcol_d,
                      cv_d, ct_d, pools, b)
    nc.compile()
    return nc


_NC_CACHE = None


def _get_nc():
    global _NC_CACHE
    if _NC_CACHE is None:
        _NC_CACHE = build_nc()
    return _NC_CACHE


def make_in_maps(vision_repr, text_repr, Wq, bq, Wk, bk, Wv, bv):
    s = 1.0 / np.sqrt(np.float64(DIM))
    wq64 = np.asarray(Wq, np.float64)
    wk64 = np.asarray(Wk, np.float64)
    # scores = vision @ A @ text.T with A = Wq.T @ Wk / sqrt(D);
    # device wants A2 = A.T = Wk.T @ Wq / sqrt(D) (contraction-major layout)
    a2 = np.ascontiguousarray((wk64.T @ wq64 * s).astype(np.float32))
    wvt = np.ascontiguousarray(np.asarray(Wv, np.float32).T)
    # surviving softmax bias term: w[t] = text @ (Wk.T @ bq) / sqrt(D)
    g = (wk64.T @ np.asarray(bq, np.float64)) * s
    txt = np.asarray(text_repr, np.float32)
    vis = np.asarray(vision_repr, np.float32)
    wcol = (txt.astype(np.float64) @ g).astype(np.float32)  # [B, T]
    in_maps = []
    for c in range(N_CORES):
        in_maps.append({
            "vision": vis[c * BPC:(c + 1) * BPC],
            "text": txt[c * BPC:(c + 1) * BPC],
            "a2": a2, "wvt": wvt,
            "wcol": wcol[c * BPC:(c + 1) * BPC],
            "ident128": np.eye(P, dtype=np.float32),
        })
    return in_maps


def kernel(vision_repr, text_repr, Wq, bq, Wk, bk, Wv, bv):
    from concourse.bass_utils import run_bass_kernel_spmd

    nc = _get_nc()
    in_maps = make_in_maps(vision_repr, text_repr, Wq, bq, Wk, bk, Wv, bv)
    res = run_bass_kernel_spmd(nc, in_maps, list(range(N_CORES))).results
    cv = np.concatenate([r_["cross_vision"] for r_ in res], axis=0)
    ct = np.concatenate([r_["cross_text"] for r_ in res], axis=0)
    cv = cv + np.asarray(bv, np.float32)[None, None, :]
    return cv, ct


# revision 12
# speedup vs baseline: 1.0377x; 1.0377x over previous
"""Cross-attention layer (vision<->text) on 8 Trainium2 NeuronCores.

Problem: B=16, Sv=St=1024, D=1024, fp32.
  q = vision @ Wq.T + bq            [B,Sv,D]
  k = text   @ Wk.T + bk            [B,St,D]
  v = text   @ Wv.T + bv            [B,St,D]
  scores = q @ k.T / sqrt(D)        [B,Sv,St]
  attn = softmax(scores, -1)
  cross_vision = attn @ v           [B,Sv,D]
  cross_text   = attn.T @ vision    [B,St,D]

Sharding: pure data-parallel over batch, 2 items per core, no collectives.

Algebraic restructure (q and k are never outputs):
  scores = vision @ A @ text.T + u 1^T + 1 w^T + c, with
    A = Wq.T @ Wk / sqrt(D)   (host-precomputed, fp64)
    u[s] = vision @ Wq.T @ bk / sqrt(D),  w[t] = text @ Wk.T @ bq / sqrt(D)
  The u[s] + c terms are constant along the softmax axis t, so they cancel
  exactly in softmax.  Only w[t] survives; host precomputes it and the device
  folds it in as the per-partition bias of the exp() (scores are built
  t-on-partitions).  bv is added on the host after gather (attn rows sum to
  1, exact).  This removes one full GEMM per item (Q-proj + K-proj + scores
  -> A-proj + scores): 5 GEMMs/item, the information-theoretic minimum.

Per-core kernel design (per batch item; all matmuls float32r, full PE rate):
  prep:  PE-transpose text -> TT[d,t], vision -> VT[d,s]   (128 transposes)
  Hproj: HT[e,t] = A2-columns.T @ TT  (A2 host-pre-swizzled for 4KB DMA rows)
  Vproj: V[t,d'] = TT.T @ wvt, TT tile stationary -> natural [t,d'] layout
  ST:    ST[t,s] = HT.T @ VT per 128-row t-tile; E2 = exp(ST + w[t]) straight
         out of PSUM on ACT (scores are O(+-6), fp32 exp needs no max-sub)
  P5:    per s-tile: PE-transpose E2 column block -> E2T[s,t] blocks, evac'd
         on ACT with accum_out giving row sums -> rinv (DVE); E2T scaled by
         rinv in place (-> attn.T rows); CV[s,:] = E2.T @ V over t-tiles,
         scaled by rinv at DVE evac
  CT:    cross_text = E2T_norm.T @ vision over s-tiles, vision streamed back

Cross-item software pipeline (PE emission order):
  prep0(txt+vis) H0 V0 ST0 P5_0(+vis-prep1 hooks) prep1(txt) CT0 H1 V1 ST1
  P5_1 CT1
  - item1's vision transposes hide inside P5_0 (VT0 is dead by then; E2/TT
    share a slot so item1's TEXT must wait until P5_0 ends);
  - item1's text tiles prefetch on the then-idle sync queue during P5_0;
  - CT0's vision tiles ride sync (dc0) / scalar (dc1) behind them.
Engine budget: PE does all matmuls/transposes; ACT does exp + E2T evacs +
H/V-proj evacs; DVE does prep/CV/CT evacs + softmax normalize; Pool engine
only drives the A2 column DMAs; stores ride the scalar queue.
"""

import sys

import numpy as np

if "/opt/trn_rl_repo" not in sys.path:
    sys.path.insert(0, "/opt/trn_rl_repo")

import concourse.bass as bass
import concourse.tile as tile
from concourse import bacc
from concourse import mybir

PHASE_MARKS = []

P = 128
B, SEQ, DIM = 16, 1024, 1024
N_CORES = 8
BPC = B // N_CORES  # batch items per core
NT = DIM // P
F32 = mybir.dt.float32
F32R = mybir.dt.float32r
AF = mybir.ActivationFunctionType
H = 512  # half of a seq dim / PSUM-bank-sized chunk


def _make_item(tc, ident, vis_d, txt_d, a2_d, wvt_d, wvt_sb, wcol_d, cv_d,
               ct_d, pools, b):
    """Emission closures for item b: prep_full / prep_vis_l / prep_text /
    mid / ct (see module docstring for the cross-item pipeline order)."""
    nc = tc.nc
    st = {}

    def mark(name):
        PHASE_MARKS.append((f"b{b}_{name}", nc._state.next_id()))

    (p_tt, p_vt, p_ht, p_v, p_wc, p_in, p_cvs, p_cts, p_vts,
     p_rp, p_rv, pp_t, pp_mm) = pools

    def prep_l(src_d, dst, eng, l, first=False):
        """One l-tile of activation transpose: [128,1024] load + 8 PE
        transposes into dst[d_in, d_out, l*128:(l+1)*128]."""
        tin = p_in.tile([P, DIM], F32R, name="tin", tag="xin")
        nch = 4 if first else 2
        W = DIM // nch
        for hh in range(nch):
            eng.dma_start(
                out=tin[:, hh * W:(hh + 1) * W],
                in_=src_d[b, l * P:(l + 1) * P, hh * W:(hh + 1) * W].bitcast(F32R))
        for tg in range(2):
            tp4 = pp_t.tile([P, 4, P], F32R, name="tp4", tag="tp4")
            for j in range(4):
                c = tg * 4 + j
                nc.tensor.matmul(
                    tp4[:, j, :], tin[:, c * P:(c + 1) * P], ident,
                    is_transpose=True, start=(j == 0), stop=(j == 3),
                    skip_group_check=True,
                )
            nc.vector.tensor_copy(
                dst[:, tg * 4:(tg + 1) * 4, l * P:(l + 1) * P], tp4)

    def _load_wcol():
        wcol_sb = p_rv.tile([P, NT], F32, name="wcol_sb", tag="wcol")
        nc.sync.dma_start(out=wcol_sb,
                          in_=wcol_d[b].rearrange("(tt ti) -> ti tt", ti=P))
        st["wcol_sb"] = wcol_sb

    def prep_full():
        """Item 0 only: text and vision interleaved per l-tile, text tiles
        on the sync queue, vision on scalar, so PE consumption order matches
        the two queues' concurrent delivery."""
        _load_wcol()
        mark("prep")
        st["tt_sb"] = p_tt.tile([P, NT, SEQ], F32R, name="tt_sb", tag="tt_e2")
        st["vt_sb"] = p_vt.tile([P, NT, SEQ], F32R, name="vt_sb", tag="vt")
        for l in range(NT):
            prep_l(txt_d, st["tt_sb"], nc.sync, l, first=(l == 0))
            prep_l(vis_d, st["vt_sb"], nc.scalar, l)
        # Wv.T rides the sync queue BEHIND item 0's text tiles: not needed
        # until projV (~100us in); putting it first would starve prep.
        nc.sync.dma_start(
            out=wvt_sb, in_=wvt_d.rearrange("(do di) e -> di do e", di=P))

    def prep_vis_l(l):
        """Items 1+: one vision l-tile, emitted from inside the PREVIOUS
        item's P5 loop (VT(b-1) is dead by then; hides the transposes)."""
        if l == 0:
            _load_wcol()
            st["vt_sb"] = p_vt.tile([P, NT, SEQ], F32R, name="vt_sb", tag="vt")
        prep_l(vis_d, st["vt_sb"], nc.scalar, l)

    def prep_text():
        """Items 1+: text transposes at the item boundary (TT shares a slot
        with E2(b-1), dead only after P5(b-1))."""
        mark("prep")
        st["tt_sb"] = p_tt.tile([P, NT, SEQ], F32R, name="tt_sb", tag="tt_e2")
        for l in range(NT):
            prep_l(txt_d, st["tt_sb"], nc.sync, l)

    def mid(vis_hook=None):
        tt_sb = st["tt_sb"]
        vt_sb = st["vt_sb"]
        wcol_sb = st["wcol_sb"]

        # ---- HT[e,t] = A2-cols.T @ TT (A2 column block streamed per eo) ----
        mark("projH")
        ht_sb = p_ht.tile([P, NT, SEQ], F32R, name="ht_sb", tag="ht_e2t")
        for eo in range(NT):
            wc = p_wc.tile([P, NT, P], F32R, name="wc", tag="wc")
            nc.gpsimd.dma_start(out=wc, in_=a2_d[eo])
            psh = [pp_mm.tile([P, H], F32, name=f"ps_h{i}", tag="mm")
                   for i in range(2)]
            for do in range(NT):
                for th in range(2):
                    nc.tensor.matmul(psh[th], wc[:, do, :],
                                     tt_sb[:, do, th * H:(th + 1) * H],
                                     start=(do == 0), stop=(do == NT - 1))
            for th in range(2):
                nc.scalar.copy(ht_sb[:, eo, th * H:(th + 1) * H], psh[th])

        # ---- V[t,d'] = TT.T @ wvt (TT stationary, wvt moving in SBUF) ----
        mark("projV")
        v_sb = p_v.tile([P, NT, SEQ], F32R, name="v_sb", tag="v")
        for tt in range(NT):
            psv = [pp_mm.tile([P, H], F32, name=f"ps_v{i}", tag="mm")
                   for i in range(2)]
            for do in range(NT):
                for dc in range(2):
                    nc.tensor.matmul(psv[dc], tt_sb[:, do, tt * P:(tt + 1) * P],
                                     wvt_sb[:, do, dc * H:(dc + 1) * H],
                                     start=(do == 0), stop=(do == NT - 1))
            for dc in range(2):
                nc.scalar.copy(v_sb[:, tt, dc * H:(dc + 1) * H], psv[dc])

        # ---- ST[t,s] = HT.T @ VT; E2 = exp(ST + w[t]) (t on partitions) ----
        mark("ST")
        e2_sb = p_tt.tile([P, NT, SEQ], F32R, name="e2_sb", tag="tt_e2")
        for tt in range(NT):
            pst = [pp_mm.tile([P, H], F32, name=f"ps_s{i}", tag="mm")
                   for i in range(2)]
            for eo in range(NT):
                for sh in range(2):
                    nc.tensor.matmul(pst[sh], ht_sb[:, eo, tt * P:(tt + 1) * P],
                                     vt_sb[:, eo, sh * H:(sh + 1) * H],
                                     start=(eo == 0), stop=(eo == NT - 1))
            for sh in range(2):
                nc.scalar.activation(out=e2_sb[:, tt, sh * H:(sh + 1) * H],
                                     in_=pst[sh], func=AF.Exp,
                                     bias=wcol_sb[:, tt:tt + 1])

        # ---- P5: per s-tile: E2T + row sums -> rinv; normalize; CV ----
        mark("P5")
        e2t_sb = p_ht.tile([P, NT, SEQ], F32R, name="e2t_sb", tag="ht_e2t")
        rinv = p_rv.tile([P, NT], F32, name="rinv", tag="rinv")
        for so in range(NT):
            rp = p_rp.tile([P, 2], F32, name="rp", tag="rp")
            for tg in range(2):
                tp4 = pp_t.tile([P, 4, P], F32R, name="tp4e", tag="tp4")
                for j in range(4):
                    tt = tg * 4 + j
                    nc.tensor.matmul(tp4[:, j, :],
                                     e2_sb[:, tt, so * P:(so + 1) * P],
                                     ident, is_transpose=True, start=(j == 0),
                                     stop=(j == 3), skip_group_check=True)
                nc.scalar.activation(out=e2t_sb[:, so, tg * H:(tg + 1) * H],
                                     in_=tp4, func=AF.Identity,
                                     accum_out=rp[:, tg:tg + 1])
            rsum = p_rp.tile([P, 1], F32, name="rsum", tag="rsum")
            nc.vector.tensor_add(rsum, rp[:, 0:1], rp[:, 1:2])
            nc.vector.reciprocal(rinv[:, so:so + 1], rsum)

            # normalize this E2T row-block in place -> attn.T rows (for CT)
            nc.vector.tensor_scalar_mul(e2t_sb[:, so, :], e2t_sb[:, so, :],
                                        scalar1=rinv[:, so:so + 1])

            # cross_vision[s-tile] = rinv * (E2.T @ V)
            cvs = p_cvs.tile([P, DIM], F32, name="cvs", tag="cvs")
            pcv = [pp_mm.tile([P, H], F32, name=f"ps_cv{i}", tag="mm")
                   for i in range(2)]
            for tt in range(NT):
                for dc in range(2):
                    nc.tensor.matmul(pcv[dc], e2_sb[:, tt, so * P:(so + 1) * P],
                                     v_sb[:, tt, dc * H:(dc + 1) * H],
                                     start=(tt == 0), stop=(tt == NT - 1))
            for dc in range(2):
                nc.vector.tensor_scalar_mul(cvs[:, dc * H:(dc + 1) * H],
                                            pcv[dc],
                                            scalar1=rinv[:, so:so + 1])
            nc.scalar.dma_start(out=cv_d[b, so * P:(so + 1) * P, :], in_=cvs)

            if vis_hook is not None:
                vis_hook(so)  # next item's vision l-tile hides here
        st["e2t_sb"] = e2t_sb

    def ct():
        e2t_sb = st["e2t_sb"]
        # ---- CT: cross_text = E2T_norm.T @ vision (vision streamed back) ---
        mark("CT")
        for dc in range(2):
            pss = [pp_mm.tile([P, H], F32, name=f"ps_ct{i}", tag="mm")
                   for i in range(6)]
            pss += [pp_t.tile([P, H], F32, name=f"ps_ct{i + 6}", tag="tp4")
                    for i in range(2)]
            for so in range(NT):
                vtl = p_vts.tile([P, H], F32R, name="vtl", tag="vtl")
                eng = nc.sync if dc == 0 else nc.scalar
                eng.dma_start(
                    out=vtl,
                    in_=vis_d[b, so * P:(so + 1) * P, dc * H:(dc + 1) * H].bitcast(F32R))
                for tt in range(NT):
                    nc.tensor.matmul(pss[tt], e2t_sb[:, so, tt * P:(tt + 1) * P],
                                     vtl, start=(so == 0), stop=(so == NT - 1))
            for tt in range(NT):
                cts = p_cts.tile([P, H], F32, name="cts", tag="cts")
                nc.vector.tensor_copy(cts, pss[tt])
                nc.scalar.dma_start(
                    out=ct_d[b, tt * P:(tt + 1) * P, dc * H:(dc + 1) * H],
                    in_=cts)
        mark("end")

    return {"prep_full": prep_full, "prep_vis_l": prep_vis_l,
            "prep_text": prep_text, "mid": mid, "ct": ct}


def build_nc():
    nc = bacc.Bacc("TRN2", target_bir_lowering=False, debug=False, num_devices=N_CORES)
    vis = nc.dram_tensor("vision", [BPC, SEQ, DIM], F32, kind="ExternalInput").ap()
    txt = nc.dram_tensor("text", [BPC, SEQ, DIM], F32, kind="ExternalInput").ap()
    a2_d = nc.dram_tensor("a2", [NT, P, NT, P], F32R, kind="ExternalInput").ap()
    wvt_d = nc.dram_tensor("wvt", [DIM, DIM], F32R, kind="ExternalInput").ap()
    wcol_d = nc.dram_tensor("wcol", [BPC, SEQ], F32, kind="ExternalInput").ap()
    id_d = nc.dram_tensor("ident128", [P, P], F32R, kind="ExternalInput").ap()
    cv_d = nc.dram_tensor("cross_vision", [BPC, SEQ, DIM], F32, kind="ExternalOutput").ap()
    ct_d = nc.dram_tensor("cross_text", [BPC, SEQ, DIM], F32, kind="ExternalOutput").ap()

    with tile.TileContext(nc) as tc:
        import contextlib
        with contextlib.ExitStack() as ctx:
            def sp(name, bufs):
                return ctx.enter_context(tc.tile_pool(name=name, bufs=bufs))

            p_tt = sp("tt", 1)    # TT then E2 (disjoint lifetimes)
            p_vt = sp("vt", 1)    # VT
            p_ht = sp("ht", 1)    # HT then E2T (disjoint lifetimes)
            p_v = sp("v", 1)      # V
            p_wvt = sp("wvt", 1)  # persistent Wv.T
            p_wc = sp("wc", 2)    # A2 column blocks
            p_in = sp("xin", 4)   # prep [128,1024] staging
            p_cvs = sp("cvs", 2)
            p_cts = sp("cts", 3)
            p_vts = sp("vtl", 4)  # CT vision tiles
            p_rp = sp("rp", 4)
            p_rv = sp("rv", 2)
            p_sm = sp("sm", 1)
            pp_t = ctx.enter_context(
                tc.tile_pool(name="pp_t", bufs=2, space=bass.MemorySpace.PSUM))
            pp_mm = ctx.enter_context(
                tc.tile_pool(name="pp_mm", bufs=6, space=bass.MemorySpace.PSUM))

            ident = p_sm.tile([P, P], F32R, name="ident")
            nc.sync.dma_start(out=ident, in_=id_d)
            wvt_sb = p_wvt.tile([P, NT, DIM], F32R, name="wvt_sb")

            pools = (p_tt, p_vt, p_ht, p_v, p_wc, p_in, p_cvs, p_cts, p_vts,
                     p_rp, p_rv, pp_t, pp_mm)
            items = [_make_item(tc, ident, vis, txt, a2_d, wvt_d, wvt_sb,
                                wcol_d, cv_d, ct_d, pools, b)
                     for b in range(BPC)]
            # PE order: prep0 H0 V0 ST0 P5_0(+vis1) prep1(txt) CT0 H1 ... CT1
            items[0]["prep_full"]()
            for b in range(BPC):
                nxt = items[b + 1] if b + 1 < BPC else None
                items[b]["mid"](nxt["prep_vis_l"] if nxt else None)
                if nxt is not None:
                    nxt["prep_text"]()
                items[b]["ct"]()
    nc.compile()
    return nc


_NC_CACHE = None


def _get_nc():
    global _NC_CACHE
    if _NC_CACHE is None:
        _NC_CACHE = build_nc()
    return _NC_CACHE


def make_in_maps(vision_repr, text_repr, Wq, bq, Wk, bk, Wv, bv):
    s = 1.0 / np.sqrt(np.float64(DIM))
    wq64 = np.asarray(Wq, np.float64)
    wk64 = np.asarray(Wk, np.float64)
    # scores = vision @ A @ text.T with A = Wq.T @ Wk / sqrt(D);
    # device wants A2 = A.T = Wk.T @ Wq / sqrt(D) (contraction-major), and
    # pre-swizzled so each per-eo column-block load has contiguous 4KB rows:
    # a2[eo][di, do, e] = A2[do*128+di, eo*128+e]
    a2 = (wk64.T @ wq64 * s).astype(np.float32)
    a2 = np.ascontiguousarray(a2.reshape(NT, P, NT, P).transpose(2, 1, 0, 3))
    wvt = np.ascontiguousarray(np.asarray(Wv, np.float32).T)
    # surviving softmax bias term: w[t] = text @ (Wk.T @ bq) / sqrt(D)
    g = (wk64.T @ np.asarray(bq, np.float64)) * s
    txt = np.asarray(text_repr, np.float32)
    vis = np.asarray(vision_repr, np.float32)
    wcol = (txt.astype(np.float64) @ g).astype(np.float32)  # [B, T]
    in_maps = []
    for c in range(N_CORES):
        in_maps.append({
            "vision": vis[c * BPC:(c + 1) * BPC],
            "text": txt[c * BPC:(c + 1) * BPC],
            "a2": a2, "wvt": wvt,
            "wcol": wcol[c * BPC:(c + 1) * BPC],
            "ident128": np.eye(P, dtype=np.float32),
        })
    return in_maps


def kernel(vision_repr, text_repr, Wq, bq, Wk, bk, Wv, bv):
    from concourse.bass_utils import run_bass_kernel_spmd

    nc = _get_nc()
    in_maps = make_in_maps(vision_repr, text_repr, Wq, bq, Wk, bk, Wv, bv)
    res = run_bass_kernel_spmd(nc, in_maps, list(range(N_CORES))).results
    cv = np.concatenate([r_["cross_vision"] for r_ in res], axis=0)
    ct = np.concatenate([r_["cross_text"] for r_ in res], axis=0)
    cv = cv + np.asarray(bv, np.float32)[None, None, :]
    return cv, ct


# revision 13
# speedup vs baseline: 1.0857x; 1.0462x over previous
"""Cross-attention layer (vision<->text) on 8 Trainium2 NeuronCores.

Problem: B=16, Sv=St=1024, D=1024, fp32.
  q = vision @ Wq.T + bq            [B,Sv,D]
  k = text   @ Wk.T + bk            [B,St,D]
  v = text   @ Wv.T + bv            [B,St,D]
  scores = q @ k.T / sqrt(D)        [B,Sv,St]
  attn = softmax(scores, -1)
  cross_vision = attn @ v           [B,Sv,D]
  cross_text   = attn.T @ vision    [B,St,D]

Sharding: pure data-parallel over batch, 2 items per core, no collectives.

Algebraic restructure (q and k are never outputs):
  scores = vision @ A @ text.T + u 1^T + 1 w^T + c, with
    A = Wq.T @ Wk / sqrt(D)   (host-precomputed, fp64)
    u[s] = vision @ Wq.T @ bk / sqrt(D),  w[t] = text @ Wk.T @ bq / sqrt(D)
  The u[s] + c terms are constant along the softmax axis t, so they cancel
  exactly in softmax.  Only w[t] survives; host precomputes it and the device
  folds it in as the per-partition bias of the exp() (scores are built
  t-on-partitions).  bv is added on the host after gather (attn rows sum to
  1, exact).  This removes one full GEMM per item (Q-proj + K-proj + scores
  -> A-proj + scores): 5 GEMMs/item, the information-theoretic minimum.

Per-core kernel design (per batch item; all matmuls float32r, full PE rate):
  prep:  PE-transpose text -> TT[d,t], vision -> VT[d,s]   (128 transposes)
  Hproj: HT[e,t] = A2-columns.T @ TT  (A2 host-pre-swizzled for 4KB DMA rows)
  Vproj: V[t,d'] = TT.T @ wvt, TT tile stationary -> natural [t,d'] layout
  ST:    ST[t,s] = HT.T @ VT per 128-row t-tile; E2 = exp(ST + w[t]) straight
         out of PSUM on ACT (scores are O(+-6), fp32 exp needs no max-sub)
  P5:    per s-tile: PE-transpose E2 column block -> E2T[s,t] blocks, evac'd
         on ACT with accum_out giving row sums -> rinv (DVE); E2T scaled by
         rinv in place (-> attn.T rows); CV[s,:] = E2.T @ V over t-tiles,
         scaled by rinv at DVE evac
  CT:    cross_text = E2T_norm.T @ vision over s-tiles, vision streamed back

Cross-item software pipeline (PE emission order):
  prep0(txt+vis) H0 V0 ST0 P5_0(+vis-prep1 hooks) prep1(txt) CT0 H1 V1 ST1
  P5_1 CT1
  - item1's vision transposes hide inside P5_0 (VT0 is dead by then; E2/TT
    share a slot so item1's TEXT must wait until P5_0 ends);
  - item1's text tiles prefetch on the then-idle sync queue during P5_0;
  - CT0's vision tiles ride sync (dc0) / scalar (dc1) behind them.
Engine budget: PE does all matmuls/transposes; ACT does exp + E2T evacs +
H/V-proj evacs; DVE does prep/CV/CT evacs + softmax normalize; Pool engine
only drives the A2 column DMAs; stores ride the scalar queue.
"""

import sys

import numpy as np

if "/opt/trn_rl_repo" not in sys.path:
    sys.path.insert(0, "/opt/trn_rl_repo")

import concourse.bass as bass
import concourse.tile as tile
from concourse import bacc
from concourse import mybir

PHASE_MARKS = []

P = 128
B, SEQ, DIM = 16, 1024, 1024
N_CORES = 8
BPC = B // N_CORES  # batch items per core
NT = DIM // P
F32 = mybir.dt.float32
F32R = mybir.dt.float32r
AF = mybir.ActivationFunctionType
H = 512  # half of a seq dim / PSUM-bank-sized chunk


def _make_item(tc, ident, vis_d, txt_d, a2_d, wvt_d, wvt_sb, wcol_d, cv_d,
               ct_d, pools, b):
    """Emission closures for item b: prep_full / prep_vis_l / prep_text /
    mid / ct (see module docstring for the cross-item pipeline order)."""
    nc = tc.nc
    st = {}

    def mark(name):
        PHASE_MARKS.append((f"b{b}_{name}", nc._state.next_id()))

    (p_tt, p_vt, p_ht, p_v, p_wc, p_in, p_cvs, p_cts, p_vts,
     p_rp, p_rv, pp_t, pp_mm) = pools

    def prep_l(src_d, dst, eng, l, first=False):
        """One l-tile of activation transpose: [128,1024] load + 8 PE
        transposes into dst[d_in, d_out, l*128:(l+1)*128]."""
        tin = p_in.tile([P, DIM], F32R, name="tin", tag="xin")
        nch = 4 if first else 2
        W = DIM // nch
        for hh in range(nch):
            eng.dma_start(
                out=tin[:, hh * W:(hh + 1) * W],
                in_=src_d[b, l * P:(l + 1) * P, hh * W:(hh + 1) * W].bitcast(F32R))
        for tg in range(2):
            tp4 = pp_t.tile([P, 4, P], F32R, name="tp4", tag="tp4")
            for j in range(4):
                c = tg * 4 + j
                nc.tensor.matmul(
                    tp4[:, j, :], tin[:, c * P:(c + 1) * P], ident,
                    is_transpose=True, start=(j == 0), stop=(j == 3),
                    skip_group_check=True,
                )
            nc.vector.tensor_copy(
                dst[:, tg * 4:(tg + 1) * 4, l * P:(l + 1) * P], tp4)

    def _load_wcol():
        wcol_sb = p_rv.tile([P, NT], F32, name="wcol_sb", tag="wcol")
        nc.sync.dma_start(out=wcol_sb,
                          in_=wcol_d[b].rearrange("(tt ti) -> ti tt", ti=P))
        st["wcol_sb"] = wcol_sb

    def prep_vis_l(l):
        """One vision l-tile; for item 0 these are emitted from inside its
        own projH loop (prep only gates on the 4MB of text), for items 1+
        from inside the PREVIOUS item's P5 loop (VT(b-1) is dead by then).
        Either way the transposes hide between matmul groups."""
        if l == 0:
            if "wcol_sb" not in st:
                _load_wcol()
            st["vt_sb"] = p_vt.tile([P, NT, SEQ], F32R, name="vt_sb", tag="vt")
        prep_l(vis_d, st["vt_sb"], nc.scalar, l)

    def prep_text():
        """Text transposes at the item boundary (TT shares a slot with
        E2(b-1), dead only after P5(b-1))."""
        if b == 0:
            _load_wcol()
        mark("prep")
        st["tt_sb"] = p_tt.tile([P, NT, SEQ], F32R, name="tt_sb", tag="tt_e2")
        for l in range(NT):
            prep_l(txt_d, st["tt_sb"], nc.sync, l, first=(b == 0 and l == 0))
        if b == 0:
            # Wv.T rides the sync queue BEHIND item 0's text tiles: not
            # needed until projV (~100us in); putting it first would starve
            # the first prep transposes of DMA bandwidth.
            nc.sync.dma_start(
                out=wvt_sb, in_=wvt_d.rearrange("(do di) e -> di do e", di=P))

    def mid(vis_hook=None, self_vis_hook=None):
        tt_sb = st["tt_sb"]
        wcol_sb = st["wcol_sb"]

        # ---- HT[e,t] = A2-cols.T @ TT (A2 column block streamed per eo) ----
        mark("projH")
        ht_sb = p_ht.tile([P, NT, SEQ], F32R, name="ht_sb", tag="ht_e2t")
        for eo in range(NT):
            wc = p_wc.tile([P, NT, P], F32R, name="wc", tag="wc")
            for dh in range(2):
                nc.gpsimd.dma_start(out=wc[:, dh * 4:(dh + 1) * 4, :],
                                    in_=a2_d[eo, :, dh * 4:(dh + 1) * 4, :])
            psh = [pp_mm.tile([P, H], F32, name=f"ps_h{i}", tag="mm")
                   for i in range(2)]
            for do in range(NT):
                for th in range(2):
                    nc.tensor.matmul(psh[th], wc[:, do, :],
                                     tt_sb[:, do, th * H:(th + 1) * H],
                                     start=(do == 0), stop=(do == NT - 1))
            for th in range(2):
                nc.scalar.copy(ht_sb[:, eo, th * H:(th + 1) * H], psh[th])
            if self_vis_hook is not None:
                self_vis_hook(eo)  # item 0's own vision l-tile hides here

        # ---- V[t,d'] = TT.T @ wvt (TT stationary, wvt moving in SBUF) ----
        mark("projV")
        v_sb = p_v.tile([P, NT, SEQ], F32R, name="v_sb", tag="v")
        for tt in range(NT):
            psv = [pp_mm.tile([P, H], F32, name=f"ps_v{i}", tag="mm")
                   for i in range(2)]
            for do in range(NT):
                for dc in range(2):
                    nc.tensor.matmul(psv[dc], tt_sb[:, do, tt * P:(tt + 1) * P],
                                     wvt_sb[:, do, dc * H:(dc + 1) * H],
                                     start=(do == 0), stop=(do == NT - 1))
            for dc in range(2):
                nc.scalar.copy(v_sb[:, tt, dc * H:(dc + 1) * H], psv[dc])

        # ---- ST[t,s] = HT.T @ VT; E2 = exp(ST + w[t]) (t on partitions) ----
        vt_sb = st["vt_sb"]
        mark("ST")
        e2_sb = p_tt.tile([P, NT, SEQ], F32R, name="e2_sb", tag="tt_e2")
        for tt in range(NT):
            pst = [pp_mm.tile([P, H], F32, name=f"ps_s{i}", tag="mm")
                   for i in range(2)]
            for eo in range(NT):
                for sh in range(2):
                    nc.tensor.matmul(pst[sh], ht_sb[:, eo, tt * P:(tt + 1) * P],
                                     vt_sb[:, eo, sh * H:(sh + 1) * H],
                                     start=(eo == 0), stop=(eo == NT - 1))
            for sh in range(2):
                nc.scalar.activation(out=e2_sb[:, tt, sh * H:(sh + 1) * H],
                                     in_=pst[sh], func=AF.Exp,
                                     bias=wcol_sb[:, tt:tt + 1])

        # ---- P5: per s-tile: E2T + row sums -> rinv; normalize; CV ----
        mark("P5")
        e2t_sb = p_ht.tile([P, NT, SEQ], F32R, name="e2t_sb", tag="ht_e2t")
        rinv = p_rv.tile([P, NT], F32, name="rinv", tag="rinv")
        for so in range(NT):
            rp = p_rp.tile([P, 2], F32, name="rp", tag="rp")
            for tg in range(2):
                tp4 = pp_t.tile([P, 4, P], F32R, name="tp4e", tag="tp4")
                for j in range(4):
                    tt = tg * 4 + j
                    nc.tensor.matmul(tp4[:, j, :],
                                     e2_sb[:, tt, so * P:(so + 1) * P],
                                     ident, is_transpose=True, start=(j == 0),
                                     stop=(j == 3), skip_group_check=True)
                nc.scalar.activation(out=e2t_sb[:, so, tg * H:(tg + 1) * H],
                                     in_=tp4, func=AF.Identity,
                                     accum_out=rp[:, tg:tg + 1])
            rsum = p_rp.tile([P, 1], F32, name="rsum", tag="rsum")
            nc.vector.tensor_add(rsum, rp[:, 0:1], rp[:, 1:2])
            nc.vector.reciprocal(rinv[:, so:so + 1], rsum)

            # normalize this E2T row-block in place -> attn.T rows (for CT)
            nc.vector.tensor_scalar_mul(e2t_sb[:, so, :], e2t_sb[:, so, :],
                                        scalar1=rinv[:, so:so + 1])

            # cross_vision[s-tile] = rinv * (E2.T @ V)
            cvs = p_cvs.tile([P, DIM], F32, name="cvs", tag="cvs")
            pcv = [pp_mm.tile([P, H], F32, name=f"ps_cv{i}", tag="mm")
                   for i in range(2)]
            for tt in range(NT):
                for dc in range(2):
                    nc.tensor.matmul(pcv[dc], e2_sb[:, tt, so * P:(so + 1) * P],
                                     v_sb[:, tt, dc * H:(dc + 1) * H],
                                     start=(tt == 0), stop=(tt == NT - 1))
            for dc in range(2):
                nc.vector.tensor_scalar_mul(cvs[:, dc * H:(dc + 1) * H],
                                            pcv[dc],
                                            scalar1=rinv[:, so:so + 1])
            nc.scalar.dma_start(out=cv_d[b, so * P:(so + 1) * P, :], in_=cvs)

            if vis_hook is not None:
                vis_hook(so)  # next item's vision l-tile hides here
        st["e2t_sb"] = e2t_sb

    def ct():
        e2t_sb = st["e2t_sb"]
        # ---- CT: cross_text = E2T_norm.T @ vision (vision streamed back) ---
        mark("CT")
        # All 16 vision-tile loads are emitted upfront so dc1's tiles prefire
        # during the dc0 pass (pool slots pace them); the first two ride the
        # then-idle scalar queue, the rest the sync queue.
        vtls = {}
        for dc in range(2):
            for so in range(NT):
                vtl = p_vts.tile([P, H], F32R, name="vtl", tag="vtl")
                eng = nc.scalar if (dc == 0 and so < 2) else nc.sync
                eng.dma_start(
                    out=vtl,
                    in_=vis_d[b, so * P:(so + 1) * P, dc * H:(dc + 1) * H].bitcast(F32R))
                vtls[(dc, so)] = vtl
        for dc in range(2):
            pss = [pp_mm.tile([P, H], F32, name=f"ps_ct{i}", tag="mm")
                   for i in range(6)]
            pss += [pp_t.tile([P, H], F32, name=f"ps_ct{i + 6}", tag="tp4")
                    for i in range(2)]
            for so in range(NT):
                vtl = vtls[(dc, so)]
                for tt in range(NT):
                    nc.tensor.matmul(pss[tt], e2t_sb[:, so, tt * P:(tt + 1) * P],
                                     vtl, start=(so == 0), stop=(so == NT - 1))
            for tt in range(NT):
                cts = p_cts.tile([P, H], F32, name="cts", tag="cts")
                nc.vector.tensor_copy(cts, pss[tt])
                nc.scalar.dma_start(
                    out=ct_d[b, tt * P:(tt + 1) * P, dc * H:(dc + 1) * H],
                    in_=cts)
        mark("end")

    return {"prep_vis_l": prep_vis_l,
            "prep_text": prep_text, "mid": mid, "ct": ct}


def build_nc():
    nc = bacc.Bacc("TRN2", target_bir_lowering=False, debug=False, num_devices=N_CORES)
    vis = nc.dram_tensor("vision", [BPC, SEQ, DIM], F32, kind="ExternalInput").ap()
    txt = nc.dram_tensor("text", [BPC, SEQ, DIM], F32, kind="ExternalInput").ap()
    a2_d = nc.dram_tensor("a2", [NT, P, NT, P], F32R, kind="ExternalInput").ap()
    wvt_d = nc.dram_tensor("wvt", [DIM, DIM], F32R, kind="ExternalInput").ap()
    wcol_d = nc.dram_tensor("wcol", [BPC, SEQ], F32, kind="ExternalInput").ap()
    id_d = nc.dram_tensor("ident128", [P, P], F32R, kind="ExternalInput").ap()
    cv_d = nc.dram_tensor("cross_vision", [BPC, SEQ, DIM], F32, kind="ExternalOutput").ap()
    ct_d = nc.dram_tensor("cross_text", [BPC, SEQ, DIM], F32, kind="ExternalOutput").ap()

    with tile.TileContext(nc) as tc:
        import contextlib
        with contextlib.ExitStack() as ctx:
            def sp(name, bufs):
                return ctx.enter_context(tc.tile_pool(name=name, bufs=bufs))

            p_tt = sp("tt", 1)    # TT then E2 (disjoint lifetimes)
            p_vt = sp("vt", 1)    # VT
            p_ht = sp("ht", 1)    # HT then E2T (disjoint lifetimes)
            p_v = sp("v", 1)      # V
            p_wvt = sp("wvt", 1)  # persistent Wv.T
            p_wc = sp("wc", 2)    # A2 column blocks
            p_in = sp("xin", 4)   # prep [128,1024] staging
            p_cvs = sp("cvs", 2)
            p_cts = sp("cts", 3)
            p_vts = sp("vtl", 4)  # CT vision tiles
            p_rp = sp("rp", 4)
            p_rv = sp("rv", 2)
            p_sm = sp("sm", 1)
            pp_t = ctx.enter_context(
                tc.tile_pool(name="pp_t", bufs=2, space=bass.MemorySpace.PSUM))
            pp_mm = ctx.enter_context(
                tc.tile_pool(name="pp_mm", bufs=6, space=bass.MemorySpace.PSUM))

            ident = p_sm.tile([P, P], F32R, name="ident")
            nc.sync.dma_start(out=ident, in_=id_d)
            wvt_sb = p_wvt.tile([P, NT, DIM], F32R, name="wvt_sb")

            pools = (p_tt, p_vt, p_ht, p_v, p_wc, p_in, p_cvs, p_cts, p_vts,
                     p_rp, p_rv, pp_t, pp_mm)
            items = [_make_item(tc, ident, vis, txt, a2_d, wvt_d, wvt_sb,
                                wcol_d, cv_d, ct_d, pools, b)
                     for b in range(BPC)]
            # PE order: prep0(txt) H0(+vis0) V0 ST0 P5_0(+vis1) prep1(txt)
            #           CT0 H1 V1 ST1 P5_1 CT1
            items[0]["prep_text"]()
            for b in range(BPC):
                nxt = items[b + 1] if b + 1 < BPC else None
                items[b]["mid"](nxt["prep_vis_l"] if nxt else None,
                                items[0]["prep_vis_l"] if b == 0 else None)
                if nxt is not None:
                    nxt["prep_text"]()
                items[b]["ct"]()
    nc.compile()
    return nc


_NC_CACHE = None


def _get_nc():
    global _NC_CACHE
    if _NC_CACHE is None:
        _NC_CACHE = build_nc()
    return _NC_CACHE


def make_in_maps(vision_repr, text_repr, Wq, bq, Wk, bk, Wv, bv):
    s = 1.0 / np.sqrt(np.float64(DIM))
    wq64 = np.asarray(Wq, np.float64)
    wk64 = np.asarray(Wk, np.float64)
    # scores = vision @ A @ text.T with A = Wq.T @ Wk / sqrt(D);
    # device wants A2 = A.T = Wk.T @ Wq / sqrt(D) (contraction-major), and
    # pre-swizzled so each per-eo column-block load has contiguous 4KB rows:
    # a2[eo][di, do, e] = A2[do*128+di, eo*128+e]
    a2 = (wk64.T @ wq64 * s).astype(np.float32)
    a2 = np.ascontiguousarray(a2.reshape(NT, P, NT, P).transpose(2, 1, 0, 3))
    wvt = np.ascontiguousarray(np.asarray(Wv, np.float32).T)
    # surviving softmax bias term: w[t] = text @ (Wk.T @ bq) / sqrt(D)
    g = (wk64.T @ np.asarray(bq, np.float64)) * s
    txt = np.asarray(text_repr, np.float32)
    vis = np.asarray(vision_repr, np.float32)
    wcol = (txt.astype(np.float64) @ g).astype(np.float32)  # [B, T]
    in_maps = []
    for c in range(N_CORES):
        in_maps.append({
            "vision": vis[c * BPC:(c + 1) * BPC],
            "text": txt[c * BPC:(c + 1) * BPC],
            "a2": a2, "wvt": wvt,
            "wcol": wcol[c * BPC:(c + 1) * BPC],
            "ident128": np.eye(P, dtype=np.float32),
        })
    return in_maps


def kernel(vision_repr, text_repr, Wq, bq, Wk, bk, Wv, bv):
    from concourse.bass_utils import run_bass_kernel_spmd

    nc = _get_nc()
    in_maps = make_in_maps(vision_repr, text_repr, Wq, bq, Wk, bk, Wv, bv)
    res = run_bass_kernel_spmd(nc, in_maps, list(range(N_CORES))).results
    cv = np.concatenate([r_["cross_vision"] for r_ in res], axis=0)
    ct = np.concatenate([r_["cross_text"] for r_ in res], axis=0)
    cv = cv + np.asarray(bv, np.float32)[None, None, :]
    return cv, ct
